# revision 20
# baseline (speedup 1.0000x reference)
"""KAN layer (B-spline + SiLU) Trainium2 kernel.

y[b,k] = scale * sum_i( silu(x_bi) W[i,k] + sum_j B_j(u_bi) C[i,k,j] ),
u = (x - g0)/h clamped to [0, 11].

The cubic B-spline basis is decomposed into one-sided truncated cubes,
side chosen per basis function so feature magnitudes stay <= 7^3 (the
f32r matmul rounds operands to ~12 mantissa bits; the classic all-left
decomposition reaches 11^3 and fails the 2e-2 gate):
  j >= 4 (left):  B_j(u) = sum_m d_m relu(u-(j+m))^3,   shifts  s in {4..10}
  j <= 3 (right): B_j(u) = sum_m d_m relu((j+4-m)-u)^3, anchors c in {1..7}
(the s=11 / c=0 cubes vanish identically on the clamped domain).

15 feature chunks (one SBUF tile each, indexed in production order) feed
a [B, 15*128] @ [15*128, 128] float32r matmul per core.

Engine split: DVE runs clamp(u) (in column halves, right behind the
split x DMA) + 10 fused relu-cube custom ops; ACT runs relu/square
chains for c=1..4 then silu; Pool (gpsimd) runs the final cube
multiplies. PSUM -> bf16 SBUF copies go on DVE/ACT in parallel.

Sharding: data-parallel over batch, 1024 rows per core on 8 cores.
"""

import math
import sys

import numpy as np

if "/opt/trn_rl_repo" not in sys.path:
    sys.path.insert(0, "/opt/trn_rl_repo")

import concourse.bass as bass  # noqa: F401
import concourse.mybir as mybir
from concourse import bacc
from concourse.tile import TileContext

B_TOTAL = 8192
IN_DIM = 128
OUT_DIM = 128
N_CORES = 8
B_CORE = B_TOTAL // N_CORES  # 1024

SL = list(range(4, 11))   # left cube shifts
CR = list(range(1, 8))    # right cube anchors
NCHUNK = 1 + len(CR) + len(SL)  # silu + 7 right + 7 left = 15
# chunk index == expected production-completion order == PE consumption order
# megas: TL1 = chunks 2..5 (s4..s7), TR = chunks 8..11 (c4..c7);
# solo s10 right after u0 (early PE start), solo s8/s9 last (small tail)
CH_L = {4: 2, 5: 3, 6: 4, 7: 5, 8: 13, 9: 14, 10: 1}
CH_R = {4: 8, 5: 9, 6: 10, 7: 11, 1: 6, 2: 7, 3: 12}
CH_SILU = 0

ACT_R = [1, 2, 3]        # right cubes via ACT relu/square + Pool mult

F32 = mybir.dt.float32
F32R = mybir.dt.float32r
BF16 = mybir.dt.bfloat16
AF = mybir.ActivationFunctionType
ALU = mybir.AluOpType

# ---------------------------------------------------------------- custom DVE ops


def _register_ops():
    from concourse.dve_ops import (
        _CUSTOM_DVE_ROW_BASE,
        _SUB_OPCODE_FOR_NAME,
        CUSTOM_DVE_SPECS,
        OPS,
        DveOp,
    )
    from concourse.dve_spec import (C0, C1, C2, PageIdx, Spec, Src0, Zero,
                                    lower, maxx, minn, relu, sq)
    from concourse.dve_uop import DveOpSpec

    def reg(name, spec):
        for op in OPS:
            if op.name == name:
                return op
        row = _CUSTOM_DVE_ROW_BASE + len(OPS)
        assert row < 0x20
        _SUB_OPCODE_FOR_NAME[name] = row
        shas = {}
        for ver in ("v3", "v4"):
            s = DveOpSpec(name=name, opcode=row, uops=lower(spec, ver=ver),
                          rd1_en=False)
            shas[ver] = s.sha(ver)
        op = DveOp(name, spec, subdim=False, uops_sha=shas)
        OPS.append(op)
        CUSTOM_DVE_SPECS[name] = spec
        return op

    # u0 = clamp(x*C0 - C1, 0, C2)
    clamp_affine = Spec(
        body=minn(maxx(Src0 * C0 - C1, Zero), C2),
        reference=lambda in0, in1, s0, s1, imm2: (
            np.minimum(np.maximum(in0 * s0 - s1, 0.0), imm2)
        ).astype(np.float32),
    )
    # left cube: relu(t)^2 * t with t = u0 + C0  (== relu(t)^3)
    _t = Src0 + C0
    cube_l = Spec(
        body=sq(relu(_t)) * _t,
        reference=lambda in0, in1, s0, s1, imm2: (
            np.maximum(in0 + s0, 0.0) ** 2 * (in0 + s0)
        ).astype(np.float32),
    )
    # right cube: t = C0 - u0
    _tr = C0 - Src0
    cube_r = Spec(
        body=sq(relu(_tr)) * _tr,
        reference=lambda in0, in1, s0, s1, imm2: (
            np.maximum(s0 - in0, 0.0) ** 2 * (s0 - in0)
        ).astype(np.float32),
    )

    # paged cubes: one instruction sweeps pages k with shift s0 + k*s1
    def _paged_ref(in0, s0, s1, sign):
        p, n = in0.shape[0], in0.shape[-1]
        sd = int(np.prod(in0.shape[1:-1]))
        x = in0.reshape(p, sd, n).astype(np.float32)
        s0v = float(s0.flat[0]) if isinstance(s0, np.ndarray) else float(s0)
        s1v = float(s1.flat[0]) if isinstance(s1, np.ndarray) else float(s1)
        sh = (s0v + np.arange(sd, dtype=np.float32) * s1v)[None, :, None]
        t = (x - sh) * sign
        return (np.maximum(t, 0.0) ** 2 * t).reshape(in0.shape).astype(np.float32)

    _pg = PageIdx(C0, C1)
    _tlp = Src0 - _pg
    cube_l_paged = Spec(
        body=sq(relu(_tlp)) * _tlp,
        reference=lambda in0, in1, s0, s1, imm2: _paged_ref(in0, s0, s1, 1.0),
    )
    _trp = _pg - Src0
    cube_r_paged = Spec(
        body=sq(relu(_trp)) * _trp,
        reference=lambda in0, in1, s0, s1, imm2: _paged_ref(in0, s0, s1, -1.0),
    )

    def reg_subdim(name, spec):
        for op in OPS:
            if op.name == name:
                return op
        row = _CUSTOM_DVE_ROW_BASE + len(OPS)
        assert row < 0x20
        _SUB_OPCODE_FOR_NAME[name] = row
        shas = {}
        for ver in ("v3", "v4"):
            s = DveOpSpec(name=name, opcode=row, uops=lower(spec, ver=ver),
                          rd1_en=False)
            shas[ver] = s.sha(ver)
        op = DveOp(name, spec, subdim=True, uops_sha=shas)
        OPS.append(op)
        CUSTOM_DVE_SPECS[name] = spec
        return op

    return (
        reg("ANT_KAN_CLAMP_AFFINE", clamp_affine),
        reg("ANT_KAN_RELU_CUBE", cube_l),
        reg("ANT_KAN_RELU_CUBE_R", cube_r),
        reg_subdim("ANT_KAN_CUBES_L_PAGED", cube_l_paged),
        reg_subdim("ANT_KAN_CUBES_R_PAGED", cube_r_paged),
    )


OP_CLAMP, OP_CUBE_L, OP_CUBE_R, OP_CUBES_LP, OP_CUBES_RP = _register_ops()

# ---------------------------------------------------------------- device kernel

_NC_CACHE = {}


def _build_nc():
    if "nc" in _NC_CACHE:
        return _NC_CACHE["nc"]
    inv_h = _NC_CACHE["inv_h"]
    g0h = _NC_CACHE["g0h"]          # g0 * inv_h  (= -5.5)
    umax = _NC_CACHE["umax"]        # 11.0

    nc = bacc.Bacc("TRN2", target_bir_lowering=False)
    xT = nc.dram_tensor("xT", [IN_DIM, B_CORE], BF16, kind="ExternalInput")
    wf = nc.dram_tensor("wf", [IN_DIM, NCHUNK, OUT_DIM], F32R, kind="ExternalInput")
    yT = nc.dram_tensor("yT", [OUT_DIM, B_CORE], BF16, kind="ExternalOutput")

    for v in (1.0, 2.0, 3.0, 4.0):
        if (F32, v) not in nc.const_aps.aps:
            t = nc.alloc_sbuf_tensor(f"const-kan-{v}", [128, 1], F32)
            nc.gpsimd.memset(t.ap(), v)
            nc.const_aps.aps[(F32, v)] = t.ap()

    with TileContext(nc) as tc:
        with (
            tc.tile_pool(name="wpool", bufs=1) as wpool,
            tc.tile_pool(name="dpool", bufs=1) as dpool,
            tc.tile_pool(name="ppool", bufs=2, space="PSUM") as ppool,
        ):
            xt = dpool.tile([IN_DIM, B_CORE], BF16, tag="xt")
            nc.sync.dma_start(out=xt[:, 0:512], in_=xT[:, 0:512])
            nc.scalar.dma_start(out=xt[:, 512:1024], in_=xT[:, 512:1024])
            wt = wpool.tile([IN_DIM, NCHUNK, OUT_DIM], F32R, tag="wt")
            nc.sync.dma_start(out=wt[:, 0:6, :], in_=wf[:, 0:6, :])
            nc.sync.dma_start(out=wt[:, 6:NCHUNK, :], in_=wf[:, 6:NCHUNK, :])

            u0 = dpool.tile([IN_DIM, B_CORE], F32, tag="u0")
            tl1 = dpool.tile([IN_DIM, 4, B_CORE], F32R, tag="tl1")  # chunks 2..5
            tr = dpool.tile([IN_DIM, 4, B_CORE], F32R, tag="tr")    # chunks 8..11
            solo = {
                j: dpool.tile([IN_DIM, B_CORE], F32R, tag=f"feat{j}", name=f"feat{j}")
                for j in (0, 1, 6, 7, 12, 13, 14)
            }

            def chunk_rhs(j, h):
                cols = slice(h * 512, (h + 1) * 512)
                if 2 <= j <= 5:
                    return tl1[:, j - 2, cols]
                if 8 <= j <= 11:
                    return tr[:, j - 8, cols]
                return solo[j][:, cols]

            # ---- DVE: clamped u in column halves (starts on first x half),
            #      then paged cube sweeps + one solo tail cube
            for h in range(2):
                nc.vector._custom_dve(OP_CLAMP, out=u0[:, h * 512:(h + 1) * 512],
                                      in0=xt[:, h * 512:(h + 1) * 512],
                                      s0=inv_h, s1=g0h, imm2=umax)
            u0b4 = u0[:, None, :].broadcast_to([IN_DIM, 4, B_CORE])
            nc.vector._custom_dve(OP_CUBE_L, out=solo[CH_L[10]][:], in0=u0[:],
                                  s0=-10.0)
            nc.vector._custom_dve(OP_CUBES_LP, out=tl1[:], in0=u0b4,
                                  s0=4.0, s1=1.0)
            nc.vector._custom_dve(OP_CUBES_RP, out=tr[:], in0=u0b4,
                                  s0=4.0, s1=1.0)
            nc.vector._custom_dve(OP_CUBE_L, out=solo[CH_L[8]][:], in0=u0[:],
                                  s0=-8.0)
            nc.vector._custom_dve(OP_CUBE_L, out=solo[CH_L[9]][:], in0=u0[:],
                                  s0=-9.0)

            # ---- ACT: silu first (ready first - only needs xt), then
            #      relu/square chains; Pool multiplies
            nc.scalar.activation(solo[CH_SILU][:], xt[:], AF.Silu)
            rt, qt = {}, {}
            for c in ACT_R:
                rt[c] = dpool.tile([IN_DIM, B_CORE], F32, tag=f"r{c}", name=f"r{c}")
                nc.scalar.activation(rt[c][:], u0[:], AF.Relu,
                                     bias=float(c), scale=-1.0)
                qt[c] = dpool.tile([IN_DIM, B_CORE], F32, tag=f"q{c}", name=f"q{c}")
                nc.scalar.activation(qt[c][:], rt[c][:], AF.Square)
                nc.gpsimd.tensor_mul(solo[CH_R[c]][:], qt[c][:], rt[c][:])

            # ---- PE: f32r matmuls in chunk-index order, one PSUM bank per
            #      column half
            ps = [ppool.tile([OUT_DIM, 512], F32, tag=f"ps{h}", name=f"ps{h}")
                  for h in range(2)]
            for j in range(NCHUNK):
                for h in range(2):
                    nc.tensor.matmul(
                        ps[h][:],
                        lhsT=wt[:, j, :],
                        rhs=chunk_rhs(j, h),
                        start=(j == 0),
                        stop=(j == NCHUNK - 1),
                    )

            # ---- PSUM -> bf16 SBUF on two engines in parallel, then DMA out
            yt = dpool.tile([OUT_DIM, B_CORE], BF16, tag="yt")
            nc.vector.tensor_scalar(yt[:, 0:512], ps[0][:], 1.0, None, ALU.mult)
            nc.sync.dma_start(out=yT[:, 0:512], in_=yt[:, 0:512])
            nc.scalar.activation(yt[:, 512:1024], ps[1][:], AF.Copy)
            nc.scalar.dma_start(out=yT[:, 512:1024], in_=yt[:, 512:1024])

    nc.finalize()
    _NC_CACHE["nc"] = nc
    return nc


# ---------------------------------------------------------------- host wrapper


def _build_weights(grid, spline_coeff, base_weight, scale):
    g0 = float(grid[0, 0])
    h = float(grid[0, 1] - grid[0, 0])
    sc = float(scale.reshape(-1)[0])
    C = np.asarray(spline_coeff, np.float64)          # [i, k, j]
    d = np.array([(-1.0) ** m * math.comb(4, m) / 6.0 for m in range(5)])

    wfull = np.zeros((IN_DIM, NCHUNK, OUT_DIM), dtype=np.float64)
    wfull[:, CH_SILU, :] = np.asarray(base_weight, np.float64)
    for j in range(8):
        for m in range(5):
            if j >= 4:
                s = j + m
                if s <= 10:
                    wfull[:, CH_L[s], :] += d[m] * C[:, :, j]
                else:
                    assert s == 11  # vanishes on clamped domain
            else:
                c = j + 4 - m
                if c >= 1:
                    wfull[:, CH_R[c], :] += d[m] * C[:, :, j]
                else:
                    assert c == 0  # vanishes on clamped domain
    return (wfull * sc).astype(np.float32), g0, h


def prepare(x, grid, spline_coeff, base_weight, scale):
    import ml_dtypes

    wfull, g0, h = _build_weights(grid, spline_coeff, base_weight, scale)
    _NC_CACHE.setdefault("inv_h", 1.0 / h)
    _NC_CACHE.setdefault("g0h", g0 / h)
    _NC_CACHE.setdefault("umax", 11.0)
    nc = _build_nc()

    xT = np.ascontiguousarray(np.asarray(x, np.float32).T)  # [128, 8192]
    in_maps = []
    for c in range(N_CORES):
        in_maps.append({
            "xT": np.ascontiguousarray(
                xT[:, c * B_CORE:(c + 1) * B_CORE]).astype(ml_dtypes.bfloat16),
            "wf": wfull,
        })
    return nc, in_maps


def assemble(results):
    yT = np.concatenate([results[c]["yT"] for c in range(N_CORES)], axis=1)
    return np.ascontiguousarray(yT.T.astype(np.float32))


def kernel(x, grid, spline_coeff, base_weight, scale):
    from concourse.bass_utils import run_bass_kernel_spmd

    nc, in_maps = prepare(x, grid, spline_coeff, base_weight, scale)
    res = run_bass_kernel_spmd(nc, in_maps, core_ids=list(range(N_CORES)))
    return assemble(res.results)


if __name__ == "__main__":
    rng = np.random.default_rng(0)
    x = rng.standard_normal((B_TOTAL, IN_DIM)).astype(np.float32)
    g = np.linspace(-1, 1, 6)
    hh = 0.4
    for _ in range(3):
        g = np.concatenate([[g[0] - hh], g, [g[-1] + hh]])
    grid = np.broadcast_to(g.astype(np.float32), (IN_DIM, 12)).copy()
    C = rng.standard_normal((IN_DIM, OUT_DIM, 8)).astype(np.float32)
    W = rng.standard_normal((IN_DIM, OUT_DIM)).astype(np.float32)
    s = np.ones((1,), np.float32)
    y = kernel(x, grid, C, W, s)
    print(y.shape, y.dtype, np.abs(y).max())


# revision 21
# speedup vs baseline: 1.0593x; 1.0593x over previous
"""KAN layer (B-spline + SiLU) Trainium2 kernel.

y[b,k] = scale * sum_i( silu(x_bi) W[i,k] + sum_j B_j(u_bi) C[i,k,j] ),
u = (x - g0)/h clamped to [0, 11].

The cubic B-spline basis is decomposed into one-sided truncated cubes,
side chosen per basis function so feature magnitudes stay <= 7^3 (the
f32r matmul rounds operands to ~12 mantissa bits; the classic all-left
decomposition reaches 11^3 and fails the 2e-2 gate):
  j >= 4 (left):  B_j(u) = sum_m d_m relu(u-(j+m))^3,   shifts  s in {4..10}
  j <= 3 (right): B_j(u) = sum_m d_m relu((j+4-m)-u)^3, anchors c in {1..7}
(the s=11 / c=0 cubes vanish identically on the clamped domain).

15 feature chunks (one SBUF tile each, indexed in production order) feed
a [B, 15*128] @ [15*128, 128] float32r matmul per core.

Engine split: DVE runs clamp(u) (in column halves, right behind the
split x DMA) + 10 fused relu-cube custom ops; ACT runs relu/square
chains for c=1..4 then silu; Pool (gpsimd) runs the final cube
multiplies. PSUM -> bf16 SBUF copies go on DVE/ACT in parallel.

Sharding: data-parallel over batch, 1024 rows per core on 8 cores.
"""

import math
import sys

import numpy as np

if "/opt/trn_rl_repo" not in sys.path:
    sys.path.insert(0, "/opt/trn_rl_repo")

import concourse.bass as bass  # noqa: F401
import concourse.mybir as mybir
from concourse import bacc
from concourse.tile import TileContext

B_TOTAL = 8192
IN_DIM = 128
OUT_DIM = 128
N_CORES = 8
B_CORE = B_TOTAL // N_CORES  # 1024

SL = list(range(4, 11))   # left cube shifts
CR = list(range(1, 8))    # right cube anchors
NCHUNK = 1 + len(CR) + len(SL)  # silu + 7 right + 7 left = 15
# chunk index == expected production-completion order == PE consumption order
# megas: TL1 = chunks 2..5 (s4..s7), TR = chunks 8..11 (c4..c7),
# TL2 = chunks 13..14 (s8, s9); solo s10 right after u0 (early PE start)
CH_L = {4: 2, 5: 3, 6: 4, 7: 5, 8: 13, 9: 14, 10: 1}
CH_R = {4: 8, 5: 9, 6: 10, 7: 11, 1: 6, 2: 7, 3: 12}
CH_SILU = 0

ACT_R = [1, 2, 3]        # right cubes via ACT relu/square + Pool mult

F32 = mybir.dt.float32
F32R = mybir.dt.float32r
BF16 = mybir.dt.bfloat16
AF = mybir.ActivationFunctionType
ALU = mybir.AluOpType

# ---------------------------------------------------------------- custom DVE ops


def _register_ops():
    from concourse.dve_ops import (
        _CUSTOM_DVE_ROW_BASE,
        _SUB_OPCODE_FOR_NAME,
        CUSTOM_DVE_SPECS,
        OPS,
        DveOp,
    )
    from concourse.dve_spec import (C0, C1, C2, PageIdx, Spec, Src0, Zero,
                                    lower, maxx, minn, relu, sq)
    from concourse.dve_uop import DveOpSpec

    def reg(name, spec):
        for op in OPS:
            if op.name == name:
                return op
        row = _CUSTOM_DVE_ROW_BASE + len(OPS)
        assert row < 0x20
        _SUB_OPCODE_FOR_NAME[name] = row
        shas = {}
        for ver in ("v3", "v4"):
            s = DveOpSpec(name=name, opcode=row, uops=lower(spec, ver=ver),
                          rd1_en=False)
            shas[ver] = s.sha(ver)
        op = DveOp(name, spec, subdim=False, uops_sha=shas)
        OPS.append(op)
        CUSTOM_DVE_SPECS[name] = spec
        return op

    # u0 = clamp(x*C0 - C1, 0, C2)
    clamp_affine = Spec(
        body=minn(maxx(Src0 * C0 - C1, Zero), C2),
        reference=lambda in0, in1, s0, s1, imm2: (
            np.minimum(np.maximum(in0 * s0 - s1, 0.0), imm2)
        ).astype(np.float32),
    )
    # left cube: relu(t)^2 * t with t = u0 + C0  (== relu(t)^3)
    _t = Src0 + C0
    cube_l = Spec(
        body=sq(relu(_t)) * _t,
        reference=lambda in0, in1, s0, s1, imm2: (
            np.maximum(in0 + s0, 0.0) ** 2 * (in0 + s0)
        ).astype(np.float32),
    )
    # right cube: t = C0 - u0
    _tr = C0 - Src0
    cube_r = Spec(
        body=sq(relu(_tr)) * _tr,
        reference=lambda in0, in1, s0, s1, imm2: (
            np.maximum(s0 - in0, 0.0) ** 2 * (s0 - in0)
        ).astype(np.float32),
    )

    # paged cubes: one instruction sweeps pages k with shift s0 + k*s1
    def _paged_ref(in0, s0, s1, sign):
        p, n = in0.shape[0], in0.shape[-1]
        sd = int(np.prod(in0.shape[1:-1]))
        x = in0.reshape(p, sd, n).astype(np.float32)
        s0v = float(s0.flat[0]) if isinstance(s0, np.ndarray) else float(s0)
        s1v = float(s1.flat[0]) if isinstance(s1, np.ndarray) else float(s1)
        sh = (s0v + np.arange(sd, dtype=np.float32) * s1v)[None, :, None]
        t = (x - sh) * sign
        return (np.maximum(t, 0.0) ** 2 * t).reshape(in0.shape).astype(np.float32)

    _pg = PageIdx(C0, C1)
    _tlp = Src0 - _pg
    cube_l_paged = Spec(
        body=sq(relu(_tlp)) * _tlp,
        reference=lambda in0, in1, s0, s1, imm2: _paged_ref(in0, s0, s1, 1.0),
    )
    _trp = _pg - Src0
    cube_r_paged = Spec(
        body=sq(relu(_trp)) * _trp,
        reference=lambda in0, in1, s0, s1, imm2: _paged_ref(in0, s0, s1, -1.0),
    )

    def reg_subdim(name, spec):
        for op in OPS:
            if op.name == name:
                return op
        row = _CUSTOM_DVE_ROW_BASE + len(OPS)
        assert row < 0x20
        _SUB_OPCODE_FOR_NAME[name] = row
        shas = {}
        for ver in ("v3", "v4"):
            s = DveOpSpec(name=name, opcode=row, uops=lower(spec, ver=ver),
                          rd1_en=False)
            shas[ver] = s.sha(ver)
        op = DveOp(name, spec, subdim=True, uops_sha=shas)
        OPS.append(op)
        CUSTOM_DVE_SPECS[name] = spec
        return op

    return (
        reg("ANT_KAN_CLAMP_AFFINE", clamp_affine),
        reg("ANT_KAN_RELU_CUBE", cube_l),
        reg("ANT_KAN_RELU_CUBE_R", cube_r),
        reg_subdim("ANT_KAN_CUBES_L_PAGED", cube_l_paged),
        reg_subdim("ANT_KAN_CUBES_R_PAGED", cube_r_paged),
    )


OP_CLAMP, OP_CUBE_L, OP_CUBE_R, OP_CUBES_LP, OP_CUBES_RP = _register_ops()

# ---------------------------------------------------------------- device kernel

_NC_CACHE = {}


def _build_nc():
    if "nc" in _NC_CACHE:
        return _NC_CACHE["nc"]
    inv_h = _NC_CACHE["inv_h"]
    g0h = _NC_CACHE["g0h"]          # g0 * inv_h  (= -5.5)
    umax = _NC_CACHE["umax"]        # 11.0

    nc = bacc.Bacc("TRN2", target_bir_lowering=False)
    xT = nc.dram_tensor("xT", [IN_DIM, B_CORE], BF16, kind="ExternalInput")
    wf = nc.dram_tensor("wf", [IN_DIM, NCHUNK, OUT_DIM], F32R, kind="ExternalInput")
    yT = nc.dram_tensor("yT", [OUT_DIM, B_CORE], BF16, kind="ExternalOutput")

    for v in (1.0, 2.0, 3.0, 4.0):
        if (F32, v) not in nc.const_aps.aps:
            t = nc.alloc_sbuf_tensor(f"const-kan-{v}", [128, 1], F32)
            nc.gpsimd.memset(t.ap(), v)
            nc.const_aps.aps[(F32, v)] = t.ap()

    with TileContext(nc) as tc:
        with (
            tc.tile_pool(name="wpool", bufs=1) as wpool,
            tc.tile_pool(name="dpool", bufs=1) as dpool,
            tc.tile_pool(name="ppool", bufs=2, space="PSUM") as ppool,
        ):
            xt = dpool.tile([IN_DIM, B_CORE], BF16, tag="xt")
            nc.sync.dma_start(out=xt[:, 0:512], in_=xT[:, 0:512])
            nc.scalar.dma_start(out=xt[:, 512:1024], in_=xT[:, 512:1024])
            wt = wpool.tile([IN_DIM, NCHUNK, OUT_DIM], F32R, tag="wt")
            nc.sync.dma_start(out=wt[:, 0:6, :], in_=wf[:, 0:6, :])
            nc.sync.dma_start(out=wt[:, 6:NCHUNK, :], in_=wf[:, 6:NCHUNK, :])

            u0 = dpool.tile([IN_DIM, B_CORE], F32, tag="u0")
            tl1 = dpool.tile([IN_DIM, 4, B_CORE], F32R, tag="tl1")  # chunks 2..5
            tr = dpool.tile([IN_DIM, 4, B_CORE], F32R, tag="tr")    # chunks 8..11
            tl2 = dpool.tile([IN_DIM, 2, B_CORE], F32R, tag="tl2")  # chunks 13..14
            solo = {
                j: dpool.tile([IN_DIM, B_CORE], F32R, tag=f"feat{j}", name=f"feat{j}")
                for j in (0, 1, 6, 7, 12)
            }

            def chunk_rhs(j, h):
                cols = slice(h * 512, (h + 1) * 512)
                if 2 <= j <= 5:
                    return tl1[:, j - 2, cols]
                if 8 <= j <= 11:
                    return tr[:, j - 8, cols]
                if 13 <= j <= 14:
                    return tl2[:, j - 13, cols]
                return solo[j][:, cols]

            # ---- DVE: clamped u in column halves (starts on first x half),
            #      then paged cube sweeps + one solo tail cube
            for h in range(2):
                nc.vector._custom_dve(OP_CLAMP, out=u0[:, h * 512:(h + 1) * 512],
                                      in0=xt[:, h * 512:(h + 1) * 512],
                                      s0=inv_h, s1=g0h, imm2=umax)
            u0b4 = u0[:, None, :].broadcast_to([IN_DIM, 4, B_CORE])
            nc.vector._custom_dve(OP_CUBE_L, out=solo[CH_L[10]][:], in0=u0[:],
                                  s0=-10.0)
            nc.vector._custom_dve(OP_CUBES_LP, out=tl1[:], in0=u0b4,
                                  s0=4.0, s1=1.0)
            nc.vector._custom_dve(OP_CUBES_RP, out=tr[:], in0=u0b4,
                                  s0=4.0, s1=1.0)
            u0b2 = u0[:, None, :].broadcast_to([IN_DIM, 2, B_CORE])
            nc.vector._custom_dve(OP_CUBES_LP, out=tl2[:], in0=u0b2,
                                  s0=8.0, s1=1.0)

            # ---- ACT: silu first (ready first - only needs xt), then
            #      relu/square chains; Pool multiplies
            nc.scalar.activation(solo[CH_SILU][:], xt[:], AF.Silu)
            rt, qt = {}, {}
            for c in ACT_R:
                rt[c] = dpool.tile([IN_DIM, B_CORE], F32, tag=f"r{c}", name=f"r{c}")
                nc.scalar.activation(rt[c][:], u0[:], AF.Relu,
                                     bias=float(c), scale=-1.0)
                qt[c] = dpool.tile([IN_DIM, B_CORE], F32, tag=f"q{c}", name=f"q{c}")
                nc.scalar.activation(qt[c][:], rt[c][:], AF.Square)
                nc.gpsimd.tensor_mul(solo[CH_R[c]][:], qt[c][:], rt[c][:])

            # ---- PE: f32r matmuls in chunk-index order, one PSUM bank per
            #      column half
            ps = [ppool.tile([OUT_DIM, 512], F32, tag=f"ps{h}", name=f"ps{h}")
                  for h in range(2)]
            for j in range(NCHUNK):
                for h in range(2):
                    nc.tensor.matmul(
                        ps[h][:],
                        lhsT=wt[:, j, :],
                        rhs=chunk_rhs(j, h),
                        start=(j == 0),
                        stop=(j == NCHUNK - 1),
                    )

            # ---- PSUM -> bf16 SBUF on two engines in parallel, then DMA out
            yt = dpool.tile([OUT_DIM, B_CORE], BF16, tag="yt")
            nc.vector.tensor_scalar(yt[:, 0:512], ps[0][:], 1.0, None, ALU.mult)
            nc.sync.dma_start(out=yT[:, 0:512], in_=yt[:, 0:512])
            nc.scalar.activation(yt[:, 512:1024], ps[1][:], AF.Copy)
            nc.scalar.dma_start(out=yT[:, 512:1024], in_=yt[:, 512:1024])

    nc.finalize()
    _NC_CACHE["nc"] = nc
    return nc


# ---------------------------------------------------------------- host wrapper


def _build_weights(grid, spline_coeff, base_weight, scale):
    g0 = float(grid[0, 0])
    h = float(grid[0, 1] - grid[0, 0])
    sc = float(scale.reshape(-1)[0])
    C = np.asarray(spline_coeff, np.float64)          # [i, k, j]
    d = np.array([(-1.0) ** m * math.comb(4, m) / 6.0 for m in range(5)])

    wfull = np.zeros((IN_DIM, NCHUNK, OUT_DIM), dtype=np.float64)
    wfull[:, CH_SILU, :] = np.asarray(base_weight, np.float64)
    for j in range(8):
        for m in range(5):
            if j >= 4:
                s = j + m
                if s <= 10:
                    wfull[:, CH_L[s], :] += d[m] * C[:, :, j]
                else:
                    assert s == 11  # vanishes on clamped domain
            else:
                c = j + 4 - m
                if c >= 1:
                    wfull[:, CH_R[c], :] += d[m] * C[:, :, j]
                else:
                    assert c == 0  # vanishes on clamped domain
    return (wfull * sc).astype(np.float32), g0, h


def prepare(x, grid, spline_coeff, base_weight, scale):
    import ml_dtypes

    wfull, g0, h = _build_weights(grid, spline_coeff, base_weight, scale)
    _NC_CACHE.setdefault("inv_h", 1.0 / h)
    _NC_CACHE.setdefault("g0h", g0 / h)
    _NC_CACHE.setdefault("umax", 11.0)
    nc = _build_nc()

    xT = np.ascontiguousarray(np.asarray(x, np.float32).T)  # [128, 8192]
    in_maps = []
    for c in range(N_CORES):
        in_maps.append({
            "xT": np.ascontiguousarray(
                xT[:, c * B_CORE:(c + 1) * B_CORE]).astype(ml_dtypes.bfloat16),
            "wf": wfull,
        })
    return nc, in_maps


def assemble(results):
    yT = np.concatenate([results[c]["yT"] for c in range(N_CORES)], axis=1)
    return np.ascontiguousarray(yT.T.astype(np.float32))


def kernel(x, grid, spline_coeff, base_weight, scale):
    from concourse.bass_utils import run_bass_kernel_spmd

    nc, in_maps = prepare(x, grid, spline_coeff, base_weight, scale)
    res = run_bass_kernel_spmd(nc, in_maps, core_ids=list(range(N_CORES)))
    return assemble(res.results)


if __name__ == "__main__":
    rng = np.random.default_rng(0)
    x = rng.standard_normal((B_TOTAL, IN_DIM)).astype(np.float32)
    g = np.linspace(-1, 1, 6)
    hh = 0.4
    for _ in range(3):
        g = np.concatenate([[g[0] - hh], g, [g[-1] + hh]])
    grid = np.broadcast_to(g.astype(np.float32), (IN_DIM, 12)).copy()
    C = rng.standard_normal((IN_DIM, OUT_DIM, 8)).astype(np.float32)
    W = rng.standard_normal((IN_DIM, OUT_DIM)).astype(np.float32)
    s = np.ones((1,), np.float32)
    y = kernel(x, grid, C, W, s)
    print(y.shape, y.dtype, np.abs(y).max())
